# revision 54
# baseline (speedup 1.0000x reference)
"""Additive (Bahdanau) attention kernel for Trainium2, data-parallel over batch.

Problem shapes (hardcoded per contract): S=128, B=16, T=64, H=256.
  outputs: (S, B, 2H) f32   encoder states
  src_len: (B,)       i64   valid source lengths
  ss:      (T, B, H)  f32   decoder states
  W1 (2H,H), b1 (H), W2 (H,H), b2 (H), v_w (H), v_b (1)

reference:
  wh = outputs @ W1 + b1          -> (B,S,H)
  ws = ss @ W2 + b2               -> (B,T,H)
  scores = tanh(wh[:,None]+ws[:,:,None]) . v_w (+v_b)   -> (B,T,S)
  masked softmax over s, then attn @ outputs -> (B,T,2H)

Sharding: batch across 8 cores (2 batches per core), weights replicated.

v2 design (ACT tanh is the per-core floor at 0.833 ns/col):
  - Inputs host-packed into 5 DMA blobs (slen, constsA f16, constsB f32,
    acts0 = outsT both batches, acts1 = outs16 both + ssT both).
  - fc1/fc2 batched across both batches on PE (f16).
  - Per (batch, s-group) slab: DVE outer-add (2x dup trick) + ACT tanh
    under tc.If(src_len > sbase) on DVE/ACT only; PE h-reduction,
    DVE psum->sbuf evict (f16) and the single-descriptor relayout DMA
    run unconditionally (big bufs=2 + unconditional g0 slabs guarantee
    stale-but-finite data on skipped slabs; masked exp zeroes them).
  - Slab order [b0g0, b1g0, b0g1, b0g2, b1g1, b1g2]: batch-0 finals
    overlap batch-1 slabs; only batch-1's final chain is tail-exposed.
  - Finals: masked exp (f16) -> den matmul (ones col) -> reciprocal ->
    f16 final matmul (attn @ outputs) -> scale -> store.
  - PE warmup on const data at program start keeps HAM at K=8/8.
v_b omitted (softmax shift-invariant); no max-subtraction (|score| <~ 13,
exp fits f32 psum; e stored f16 is safe for this data regime: scores
concentrate in [-4, 4]).
"""

import os
import numpy as np
from contextlib import ExitStack

import concourse.bass as bass
import concourse.bacc as bacc
import concourse.tile as tile
from concourse import mybir
from concourse.bass_utils import run_bass_kernel_spmd


S, B, T, H = 128, 16, 64, 256
E = 2 * H
NCORES = 8
BPC = B // NCORES          # batches per core
SGROUPS = [48, 48, 32]     # s rows per group (6/6/4 psum chunks)
F32, F16, F8 = mybir.dt.float32, mybir.dt.float16, mybir.dt.float8e4
I32 = mybir.dt.int32

# w1b f16 column layout: w1 (4 chunks x 256) | vrep (2x32) | ones
W1B_VREP, W1B_ONES = 1024, 1088
W1B_COLS = 1089
# w2b f16: w2 (2 chunks x 256)
W2B_COLS = 512
# constsB f32 column layout: b1(2) b2(2) maskb(2)
CB_COLS = 6
# acts0: outsT interleaved (c, b, s) -> 4*2*128 = 1024 cols
# acts1: outs16 b0 (512) | outs16 b1 (512) | ssT (2 chunks x (b,t) 128) = 1280
A1_OUTS, A1_SS = 0, 1024

_prog_cache = {}


def build_program():
    nc = bacc.Bacc("TRN2", target_bir_lowering=False, debug=False,
                   num_devices=NCORES)

    slen_d = nc.declare_dram_parameter("slen", [BPC], I32, isOutput=False)
    w1b_d = nc.declare_dram_parameter("w1b", [128, W1B_COLS], F16, isOutput=False)
    w2b_d = nc.declare_dram_parameter("w2b", [128, W2B_COLS], F16, isOutput=False)
    cb_d = nc.declare_dram_parameter("constsB", [128, CB_COLS], F32, isOutput=False)
    a0_d = nc.declare_dram_parameter("acts0", [128, 4, BPC, S], F16, isOutput=False)
    a1_d = nc.declare_dram_parameter("acts1", [128, A1_SS + 2 * BPC * T], F16,
                                     isOutput=False)
    out_d = nc.declare_dram_parameter("out", [BPC, T, E], F32, isOutput=True)

    no_warm = bool(os.environ.get("KERNEL_NO_WARM"))
    no_if = bool(os.environ.get("KERNEL_NO_IF"))
    no_fill = bool(os.environ.get("KERNEL_NO_FILL"))

    with ExitStack() as ctx:
        tc = ctx.enter_context(tile.TileContext(nc))
        consts = ctx.enter_context(tc.tile_pool(name="consts", bufs=1))
        work = ctx.enter_context(tc.tile_pool(name="work", bufs=2))
        bigp = ctx.enter_context(tc.tile_pool(name="bigp", bufs=3))
        smallp = ctx.enter_context(tc.tile_pool(name="smallp", bufs=2))
        fc_ps = ctx.enter_context(tc.tile_pool(name="fc_ps", bufs=2, space="PSUM"))
        scq_ps = ctx.enter_context(tc.tile_pool(name="scq_ps", bufs=2, space="PSUM"))
        out_ps = ctx.enter_context(tc.tile_pool(name="out_ps", bufs=1, space="PSUM"))

        # ---- input DMAs on both queues; fc deps split so the two fc
        # input streams land in parallel ----
        slen_sb = consts.tile([1, BPC], I32)
        nc.sync.dma_start(slen_sb, slen_d[:])
        cb_sb = consts.tile([128, CB_COLS], F32)
        nc.sync.dma_start(cb_sb, cb_d[:])
        w1b_sb = consts.tile([128, W1B_COLS], F16)
        nc.sync.dma_start(w1b_sb, w1b_d[:])
        a0_sb = consts.tile([128, 4, BPC, S], F16)   # [e%128, c, b, s]
        nc.sync.dma_start(a0_sb, a0_d[:])
        w2b_sb = consts.tile([128, W2B_COLS], F16)
        nc.gpsimd.dma_start(w2b_sb, w2b_d[:])
        a1_sb = consts.tile([128, A1_SS + 2 * BPC * T], F16)
        nc.gpsimd.dma_start(a1_sb, a1_d[:])

        w1v = w1b_sb[:, 0:1024].rearrange("p (c m) -> p c m", c=4)
        vrep = w1b_sb[:, W1B_VREP:W1B_VREP + 64].rearrange(
            "p (m k) -> p m k", m=2)
        ones16 = w1b_sb[:, W1B_ONES:W1B_ONES + 1]
        w2v = w2b_sb[:, 0:512].rearrange("p (c m) -> p c m", c=2)
        ssv = a1_sb[:, A1_SS:].rearrange("p (c bt) -> p c bt", c=2)
        outs16 = [a1_sb[:, A1_OUTS + b * E:A1_OUTS + (b + 1) * E]
                  for b in range(BPC)]

        # ---- PE warmup on const data: HAM needs ~3.4us of busy to reach
        # K=8/8; run it during the framework preamble / input-DMA wait ----
        # 2048-cycle f32 matmuls: warmups cross the 3.4us HAM window so the
        # clock reaches K=8/8 before fc1; fills keep it there.
        warmt = consts.tile([128, 512], F32)
        if not no_warm:
            nc.vector.memset(warmt, 1.0)
            wps = out_ps.tile([T, 512], F32, tag="den")
            for _ in range(4):
                nc.tensor.matmul(wps, warmt[:, 0:T], warmt,
                                 start=True, stop=True, skip_group_check=True)

        # e_sb zero-init: skipped slabs' rows stay 0 (= masked weight); the
        # per-slab If-guarded exp only writes covered rows, so nothing ever
        # consumes the unconditional DMAs of skipped slabs.
        scoresT = [None] * BPC
        e_sb = [None] * BPC
        for b in range(BPC):
            scoresT[b] = smallp.tile([S, T], F16, tag="scoresT", name=f"scT{b}")
            e_sb[b] = smallp.tile([S, T], F16, tag="e_sb", name=f"e{b}")
            nc.gpsimd.memset(e_sb[b], 0.0)

        rv = [None] * BPC      # DVE+ACT: outer-add + tanh Ifs
        rv_p = [None] * BPC    # PE: reduction Ifs
        rv_e = [None] * BPC    # ACT: exp If
        for b in range(BPC):
            rv[b] = nc.values_load(
                slen_sb[0:1, b:b + 1], min_val=1, max_val=S,
                skip_runtime_bounds_check=True,
                engines=(mybir.EngineType.DVE, mybir.EngineType.Activation))
            rv_p[b] = nc.values_load(
                slen_sb[0:1, b:b + 1], min_val=1, max_val=S,
                skip_runtime_bounds_check=True,
                engines=(mybir.EngineType.PE,))
            rv_e[b] = nc.values_load(
                slen_sb[0:1, b:b + 1], min_val=1, max_val=S,
                skip_runtime_bounds_check=True,
                engines=(mybir.EngineType.Activation,))

        # ---- fc1 + fc2 (both batches), interleaved per m-half so the
        # DVE chain for the first slab (whdup b0 m0, wsT m0) is shortest ----
        whdup = [None] * BPC
        for b in range(BPC):
            whdup[b] = work.tile([128, 2, 2 * S], F16, tag="whdup",
                                 name=f"whdup{b}")
        wsT = consts.tile([128, 2, BPC * T], F16)
        for m in range(2):
            ps1 = fc_ps.tile([128, BPC * S], F32, tag="fc")
            for c in range(4):
                nc.tensor.matmul(ps1, w1v[:, c, m * 128:(m + 1) * 128],
                                 a0_sb[:, c, :, :].rearrange("p b s -> p (b s)"),
                                 start=(c == 0), stop=(c == 3))
            ps2 = fc_ps.tile([128, BPC * T], F32, tag="fc")
            for c in range(2):
                nc.tensor.matmul(ps2, w2v[:, c, m * 128:(m + 1) * 128],
                                 ssv[:, c, :], start=(c == 0), stop=(c == 1))
            for b in range(BPC):
                src = ps1[:, b * S:(b + 1) * S]
                dst = whdup[b][:, m, :].rearrange("p (s two) -> p s two", two=2)
                nc.vector.tensor_scalar_add(
                    dst, src[:, :, None].broadcast_to([128, S, 2]),
                    cb_sb[:, m:m + 1])
                if b == 0:
                    nc.vector.tensor_scalar_add(wsT[:, m, :], ps2,
                                                cb_sb[:, 2 + m:3 + m])

        # ---- slab pipeline: batch-contiguous order so b0's finals
        # overlap b1's compute ----
        sbases = np.cumsum([0] + SGROUPS)[:-1].tolist()
        order = [(b, g) for b in range(BPC) for g in range(len(SGROUPS))]

        def emit_finals(b):
            dps = out_ps.tile([T, 256], F32, tag="den")
            nc.tensor.matmul(dps[:, 0:1], e_sb[b], ones16,
                             start=True, stop=True, skip_group_check=True)
            rden = smallp.tile([T, 1], F32, tag="rden")
            nc.vector.reciprocal(rden, dps[:, 0:1])
            ops = out_ps.tile([T, E], F32, tag="ops")
            nc.tensor.matmul(ops, e_sb[b], outs16[b], start=True, stop=True)
            res = work.tile([T, E], F32, tag="res")
            nc.vector.tensor_scalar_mul(res, ops, rden)
            nc.sync.dma_start(out_d[b], res)

        def slab_addtanh(b, g, sbase, scount, big):
            for m in range(2):
                in0 = (whdup[b][:, m, 2 * sbase:2 * (sbase + scount)]
                       .rearrange("p (s two) -> p s two", two=2)
                       [:, :, None, :].broadcast_to([128, scount, 32, 2]))
                in1 = (wsT[:, m, b * T:(b + 1) * T]
                       .rearrange("p (t2 two) -> p t2 two", two=2)
                       [:, None, :, :].broadcast_to([128, scount, 32, 2]))
                oap = big[:, m, 0:64 * scount].rearrange(
                    "p (s t2 two) -> p s t2 two", s=scount, two=2)
                nc.vector.tensor_tensor(oap, in0, in1, op=mybir.AluOpType.add)
                nc.scalar.activation(big[:, m, 0:64 * scount],
                                     big[:, m, 0:64 * scount],
                                     mybir.ActivationFunctionType.Tanh)

        def slab_reduce(scount, big, scq):
            # m-outer: the m=0 pass streams while tanh m=1 still runs.
            # chunk k (8 s) -> psum rows 32*(k//2), col half 512*(k%2)
            for m in range(2):
                for k in range(scount // 8):
                    r, ch = 32 * (k // 2), 512 * (k % 2)
                    nc.tensor.matmul(scq[r:r + 32, ch:ch + 512],
                                     vrep[:, m, :],
                                     big[:, m, 512 * k:512 * (k + 1)],
                                     start=(m == 0), stop=(m == 1),
                                     skip_group_check=True)

        def emit_fill(n=1):
            if no_fill:
                return
            for _ in range(n):
                fill = out_ps.tile([T, 512], F32, tag="den")
                nc.tensor.matmul(fill, warmt[:, 0:T], warmt,
                                 start=True, stop=True, skip_group_check=True)

        # bridge the PE gap between fc and the first reduction (HAM would
        # re-throttle after 3.4us idle)
        emit_fill(8)

        def flush(pb, pg, psb, psc, pscq):
            k1 = (psc // 8 + 1) // 2     # row groups (3 for 48, 2 for 32)
            scr = smallp.tile([96, 1024], F16, tag="scr")
            nc.vector.tensor_copy(scr[0:32 * k1, :], pscq[0:32 * k1, :])
            # flat dest (one partition dim); src flatten (k1,k2,slo,t)
            # matches dest s-local = 16*k1 + 8*k2 + slo
            src = scr[0:32 * k1:32, :].rearrange(
                "k1 (k2 slo t) -> k1 k2 slo t", k2=2, slo=8)
            nc.sync.dma_start(scoresT[pb][psb:psb + psc, :], src)
            # masked exp once rows are in place: after g1 the whole [0:96)
            # block (g0+g1) unconditionally -- skipped g1 rows hold stale
            # finite scores that the -1e30 mask bias zeroes; after g2 the
            # [96:128) block under its own If.
            if pg == 1:
                rows = slice(0, 96)
                nc.scalar.activation(e_sb[pb][rows, :], scoresT[pb][rows, :],
                                     mybir.ActivationFunctionType.Exp,
                                     bias=cb_sb[rows, 4 + pb:5 + pb])
            elif pg == 2:
                rows = slice(96, 128)
                if no_if:
                    nc.scalar.activation(e_sb[pb][rows, :],
                                         scoresT[pb][rows, :],
                                         mybir.ActivationFunctionType.Exp,
                                         bias=cb_sb[rows, 4 + pb:5 + pb])
                else:
                    with tc.If(rv_e[pb] > 96):
                        nc.scalar.activation(e_sb[pb][rows, :],
                                             scoresT[pb][rows, :],
                                             mybir.ActivationFunctionType.Exp,
                                             bias=cb_sb[rows, 4 + pb:5 + pb])
                emit_finals(pb)

        pending = []
        for i, (b, g) in enumerate(order):
            sbase, scount = sbases[g], SGROUPS[g]
            big = bigp.tile([128, 2, 64 * SGROUPS[0]], F16, tag="big")
            # adds+tanh first so they never queue behind the evict on DVE
            if g == 0 or no_if:
                slab_addtanh(b, g, sbase, scount, big)
            else:
                with tc.If(rv[b] > sbase):
                    slab_addtanh(b, g, sbase, scount, big)
            # flush slab i-2: its reduction finished ~2 slabs ago, and the
            # scq ring slot reader is known before slab i reallocates it
            if len(pending) >= 2:
                flush(*pending.pop(0))
            scq = scq_ps.tile([96, 1024], F32, tag="scq", name=f"scq{b}g{g}")
            if g == 0 or no_if:
                slab_reduce(scount, big, scq)
            else:
                with tc.If(rv_p[b] > sbase):
                    slab_reduce(scount, big, scq)
            emit_fill(1)
            pending.append((b, g, sbase, scount, scq))
        for pend in pending:
            flush(*pend)

    nc.finalize()
    return nc


def _get_program():
    if "nc" not in _prog_cache:
        _prog_cache["nc"] = build_program()
    return _prog_cache["nc"]


def _balanced_assignment(src_len):
    """Pair batches so per-core work is balanced (min-max coverage).

    Returns perm: perm[c*BPC+j] = original batch index at core c slot j.
    Slot 1 (the tail-exposed batch) gets the smaller coverage of the pair.
    """
    bounds = np.cumsum([0] + SGROUPS)
    cov = [int(bounds[np.searchsorted(bounds, l)]) for l in src_len]
    order = sorted(range(B), key=lambda i: -cov[i])
    loads = [0] * NCORES
    slots = [[] for _ in range(NCORES)]
    for i in order:
        c = min((k for k in range(NCORES) if len(slots[k]) < BPC),
                key=lambda k: loads[k])
        slots[c].append(i)
        loads[c] += cov[i]
    for sl in slots:
        sl.sort(key=lambda i: -cov[i])   # slot order: [bigger, smaller]
    return [i for sl in slots for i in sl]


def make_in_maps(outputs, src_len, ss, W1, b1, W2, b2, v_w, v_b):
    outputs = np.asarray(outputs, dtype=np.float32)
    ss = np.asarray(ss, dtype=np.float32)
    src_len = np.asarray(src_len).astype(np.int64)
    perm = _balanced_assignment(src_len)
    maskb = np.where(np.arange(S)[None, :] < src_len[:, None],
                     np.float32(0.0), np.float32(-1e30)).astype(np.float32)

    # shared weight blobs
    w1b = np.zeros((128, W1B_COLS), dtype=np.float16)
    w1b[:, 0:1024] = np.asarray(W1, np.float16).reshape(4, 128, H) \
        .transpose(1, 0, 2).reshape(128, 1024)
    vr = np.repeat(np.asarray(v_w, np.float16).reshape(2, 128)
                   .transpose(1, 0)[:, :, None], 32, axis=2)   # [128, 2, 32]
    w1b[:, W1B_VREP:W1B_VREP + 64] = vr.reshape(128, 64)
    w1b[:, W1B_ONES] = np.float16(1.0)
    w2b = np.ascontiguousarray(np.asarray(W2, np.float16).reshape(2, 128, H)
                               .transpose(1, 0, 2).reshape(128, 512))

    b1_32 = np.asarray(b1, np.float32).reshape(2, 128).T     # [128, 2]
    b2_32 = np.asarray(b2, np.float32).reshape(2, 128).T

    oT16 = np.ascontiguousarray(
        outputs.transpose(1, 2, 0).astype(np.float16))        # (B, E, S)
    o16 = np.ascontiguousarray(
        outputs.transpose(1, 0, 2).astype(np.float16))        # (B, S, E)
    sT16 = np.ascontiguousarray(ss.transpose(1, 2, 0).astype(np.float16))  # (B,H,T)

    in_maps = []
    for c in range(NCORES):
        idx = perm[c * BPC:(c + 1) * BPC]
        cb = np.zeros((128, CB_COLS), dtype=np.float32)
        cb[:, 0:2] = b1_32
        cb[:, 2:4] = b2_32
        for j, bi in enumerate(idx):
            cb[:, 4 + j] = maskb[bi]
        # acts0: [e%128, c, b, s]
        a0 = oT16[idx].reshape(BPC, 4, 128, S).transpose(2, 1, 0, 3)
        # acts1: outs16 per batch [s, e] then ssT [h%128, c, (b,t)]
        a1 = np.zeros((128, A1_SS + 2 * BPC * T), dtype=np.float16)
        for j, bi in enumerate(idx):
            a1[:, A1_OUTS + j * E:A1_OUTS + (j + 1) * E] = o16[bi]
        a1[:, A1_SS:] = sT16[idx].reshape(BPC, 2, 128, T) \
            .transpose(2, 1, 0, 3).reshape(128, 2 * BPC * T)
        in_maps.append({
            "slen": np.ascontiguousarray(src_len[idx].astype(np.int32)),
            "w1b": w1b,
            "w2b": w2b,
            "constsB": cb,
            "acts0": np.ascontiguousarray(a0),
            "acts1": a1,
        })
    return in_maps, perm


def run(in_maps, trace=False, **kw):
    nc = _get_program()
    return run_bass_kernel_spmd(nc, in_maps, list(range(NCORES)), trace=trace, **kw)


def kernel(outputs, src_len, ss, W1, b1, W2, b2, v_w, v_b):
    in_maps, perm = make_in_maps(outputs, src_len, ss, W1, b1, W2, b2, v_w, v_b)
    res = run(in_maps)
    shuffled = np.concatenate([np.asarray(r["out"]).reshape(BPC, T, E)
                               for r in res.results], axis=0)
    out = np.empty_like(shuffled)
    out[np.asarray(perm)] = shuffled
    return out.astype(np.float32)  # (B, T, 2H)


# revision 60
# speedup vs baseline: 1.0188x; 1.0188x over previous
"""Additive (Bahdanau) attention kernel for Trainium2, data-parallel over batch.

Problem shapes (hardcoded per contract): S=128, B=16, T=64, H=256.
  outputs: (S, B, 2H) f32   encoder states
  src_len: (B,)       i64   valid source lengths
  ss:      (T, B, H)  f32   decoder states
  W1 (2H,H), b1 (H), W2 (H,H), b2 (H), v_w (H), v_b (1)

reference:
  wh = outputs @ W1 + b1          -> (B,S,H)
  ws = ss @ W2 + b2               -> (B,T,H)
  scores = tanh(wh[:,None]+ws[:,:,None]) . v_w (+v_b)   -> (B,T,S)
  masked softmax over s, then attn @ outputs -> (B,T,2H)

Sharding: batch across 8 cores (2 batches per core), weights replicated.

v2 design (ACT tanh is the per-core floor at 0.833 ns/col):
  - Inputs host-packed into 5 DMA blobs (slen, constsA f16, constsB f32,
    acts0 = outsT both batches, acts1 = outs16 both + ssT both).
  - fc1/fc2 batched across both batches on PE (f16).
  - Per (batch, s-group) slab: DVE outer-add (2x dup trick) + ACT tanh
    under tc.If(src_len > sbase) on DVE/ACT only; PE h-reduction,
    DVE psum->sbuf evict (f16) and the single-descriptor relayout DMA
    run unconditionally (big bufs=2 + unconditional g0 slabs guarantee
    stale-but-finite data on skipped slabs; masked exp zeroes them).
  - Slab order [b0g0, b1g0, b0g1, b0g2, b1g1, b1g2]: batch-0 finals
    overlap batch-1 slabs; only batch-1's final chain is tail-exposed.
  - Finals: masked exp (f16) -> den matmul (ones col) -> reciprocal ->
    f16 final matmul (attn @ outputs) -> scale -> store.
  - PE warmup on const data at program start keeps HAM at K=8/8.
v_b omitted (softmax shift-invariant); no max-subtraction (|score| <~ 13,
exp fits f32 psum; e stored f16 is safe for this data regime: scores
concentrate in [-4, 4]).
"""

import os
import numpy as np
from contextlib import ExitStack

import concourse.bass as bass
import concourse.bacc as bacc
import concourse.tile as tile
from concourse import mybir
from concourse.bass_utils import run_bass_kernel_spmd


S, B, T, H = 128, 16, 64, 256
E = 2 * H
NCORES = 8
BPC = B // NCORES          # batches per core
SGROUPS = [48, 48, 32]     # s rows per group (6/6/4 psum chunks)
F32, F16, F8 = mybir.dt.float32, mybir.dt.float16, mybir.dt.float8e4
I32 = mybir.dt.int32

# w1b f16 column layout: w1 (4 chunks x 256) | vrep (2x32) | ones
W1B_VREP, W1B_ONES = 1024, 1088
W1B_COLS = 1089
# w2b f16: w2 (2 chunks x 256)
W2B_COLS = 512
# constsB f32 column layout: b1(2) b2(2) maskb(2)
CB_COLS = 6
# acts0: outsT interleaved (c, b, s) -> 4*2*128 = 1024 cols
# acts1: outs16 b0 (512) | outs16 b1 (512)
# ssb: ssT (2 chunks x (b,t)) = 256 cols

_prog_cache = {}


def build_program():
    nc = bacc.Bacc("TRN2", target_bir_lowering=False, debug=False,
                   num_devices=NCORES)

    slen_d = nc.declare_dram_parameter("slen", [BPC], I32, isOutput=False)
    w1b_d = nc.declare_dram_parameter("w1b", [128, W1B_COLS], F16, isOutput=False)
    w2b_d = nc.declare_dram_parameter("w2b", [128, W2B_COLS], F16, isOutput=False)
    cb_d = nc.declare_dram_parameter("constsB", [128, CB_COLS], F32, isOutput=False)
    a0_d = nc.declare_dram_parameter("acts0", [128, 4, BPC, S], F16, isOutput=False)
    a1_d = nc.declare_dram_parameter("acts1", [128, BPC * E], F16, isOutput=False)
    ss_d = nc.declare_dram_parameter("ssb", [128, 2 * BPC * T], F16, isOutput=False)
    out_d = nc.declare_dram_parameter("out", [BPC, T, E], F32, isOutput=True)

    no_warm = bool(os.environ.get("KERNEL_NO_WARM"))
    no_if = bool(os.environ.get("KERNEL_NO_IF"))
    no_fill = bool(os.environ.get("KERNEL_NO_FILL"))

    with ExitStack() as ctx:
        tc = ctx.enter_context(tile.TileContext(nc))
        consts = ctx.enter_context(tc.tile_pool(name="consts", bufs=1))
        work = ctx.enter_context(tc.tile_pool(name="work", bufs=2))
        bigp = ctx.enter_context(tc.tile_pool(name="bigp", bufs=3))
        smallp = ctx.enter_context(tc.tile_pool(name="smallp", bufs=2))
        fc_ps = ctx.enter_context(tc.tile_pool(name="fc_ps", bufs=2, space="PSUM"))
        scq_ps = ctx.enter_context(tc.tile_pool(name="scq_ps", bufs=2, space="PSUM"))
        out_ps = ctx.enter_context(tc.tile_pool(name="out_ps", bufs=1, space="PSUM"))

        # ---- input DMAs on both queues, ordered so both fc input
        # streams (acts0+w1b | ssb+w2b) land as early as possible;
        # outs16 (finals-only) trails ----
        slen_sb = consts.tile([1, BPC], I32)
        nc.sync.dma_start(slen_sb, slen_d[:])
        cb_sb = consts.tile([128, CB_COLS], F32)
        nc.sync.dma_start(cb_sb, cb_d[:])
        a0_sb = consts.tile([128, 4, BPC, S], F16)   # [e%128, c, b, s]
        nc.sync.dma_start(a0_sb, a0_d[:])
        w1b_sb = consts.tile([128, W1B_COLS], F16)
        nc.gpsimd.dma_start(w1b_sb, w1b_d[:])
        ss_sb = consts.tile([128, 2 * BPC * T], F16)
        nc.gpsimd.dma_start(ss_sb, ss_d[:])
        w2b_sb = consts.tile([128, W2B_COLS], F16)
        nc.gpsimd.dma_start(w2b_sb, w2b_d[:])
        a1_sb = consts.tile([128, BPC * E], F16)
        nc.sync.dma_start(a1_sb, a1_d[:])

        w1v = w1b_sb[:, 0:1024].rearrange("p (c m) -> p c m", c=4)
        vrep = w1b_sb[:, W1B_VREP:W1B_VREP + 64].rearrange(
            "p (m k) -> p m k", m=2)
        ones16 = w1b_sb[:, W1B_ONES:W1B_ONES + 1]
        w2v = w2b_sb[:, 0:512].rearrange("p (c m) -> p c m", c=2)
        ssv = ss_sb.rearrange("p (c bt) -> p c bt", c=2)
        outs16 = [a1_sb[:, b * E:(b + 1) * E] for b in range(BPC)]

        # ---- PE warmup on const data: HAM needs ~3.4us of busy to reach
        # K=8/8; run it during the framework preamble / input-DMA wait ----
        # 2048-cycle f32 matmuls: warmups cross the 3.4us HAM window so the
        # clock reaches K=8/8 before fc1; fills keep it there.
        warmt = consts.tile([128, 512], F32)
        if not no_warm:
            nc.vector.memset(warmt, 1.0)
            wps = out_ps.tile([T, 512], F32, tag="den")
            for _ in range(4):
                nc.tensor.matmul(wps, warmt[:, 0:T], warmt,
                                 start=True, stop=True, skip_group_check=True)

        # e_sb zero-init: skipped slabs' rows stay 0 (= masked weight); the
        # per-slab If-guarded exp only writes covered rows, so nothing ever
        # consumes the unconditional DMAs of skipped slabs.
        scoresT = [None] * BPC
        e_sb = [None] * BPC
        for b in range(BPC):
            scoresT[b] = smallp.tile([S, T], F16, tag="scoresT", name=f"scT{b}")
            e_sb[b] = smallp.tile([S, T], F16, tag="e_sb", name=f"e{b}")
            nc.gpsimd.memset(e_sb[b], 0.0)

        rv = [None] * BPC      # DVE+ACT: outer-add + tanh Ifs
        rv_p = [None] * BPC    # PE: reduction Ifs
        rv_e = [None] * BPC    # ACT: exp If
        for b in range(BPC):
            rv[b] = nc.values_load(
                slen_sb[0:1, b:b + 1], min_val=1, max_val=S,
                skip_runtime_bounds_check=True,
                engines=(mybir.EngineType.DVE, mybir.EngineType.Activation))
            rv_p[b] = nc.values_load(
                slen_sb[0:1, b:b + 1], min_val=1, max_val=S,
                skip_runtime_bounds_check=True,
                engines=(mybir.EngineType.PE,))
            rv_e[b] = nc.values_load(
                slen_sb[0:1, b:b + 1], min_val=1, max_val=S,
                skip_runtime_bounds_check=True,
                engines=(mybir.EngineType.Activation,))

        # ---- fc1 + fc2 (both batches), interleaved per m-half so the
        # DVE chain for the first slab (whdup b0 m0, wsT m0) is shortest ----
        whdup = [None] * BPC
        for b in range(BPC):
            whdup[b] = work.tile([128, 2, 2 * S], F16, tag="whdup",
                                 name=f"whdup{b}")
        wsT = consts.tile([128, 2, BPC * T], F16)
        for m in range(2):
            ps1 = fc_ps.tile([128, BPC * S], F32, tag="fc")
            for c in range(4):
                nc.tensor.matmul(ps1, w1v[:, c, m * 128:(m + 1) * 128],
                                 a0_sb[:, c, :, :].rearrange("p b s -> p (b s)"),
                                 start=(c == 0), stop=(c == 3))
            ps2 = fc_ps.tile([128, BPC * T], F32, tag="fc")
            for c in range(2):
                nc.tensor.matmul(ps2, w2v[:, c, m * 128:(m + 1) * 128],
                                 ssv[:, c, :], start=(c == 0), stop=(c == 1))
            for b in range(BPC):
                src = ps1[:, b * S:(b + 1) * S]
                dst = whdup[b][:, m, :].rearrange("p (s two) -> p s two", two=2)
                nc.vector.tensor_scalar_add(
                    dst, src[:, :, None].broadcast_to([128, S, 2]),
                    cb_sb[:, m:m + 1])
                if b == 0:
                    nc.vector.tensor_scalar_add(wsT[:, m, :], ps2,
                                                cb_sb[:, 2 + m:3 + m])

        # ---- slab pipeline: batch-contiguous order so b0's finals
        # overlap b1's compute ----
        sbases = np.cumsum([0] + SGROUPS)[:-1].tolist()
        order = [(b, g) for b in range(BPC) for g in range(len(SGROUPS))]

        def emit_finals(b):
            dps = out_ps.tile([T, 256], F32, tag="den")
            nc.tensor.matmul(dps[:, 0:1], e_sb[b], ones16,
                             start=True, stop=True, skip_group_check=True)
            rden = smallp.tile([T, 1], F32, tag="rden")
            nc.vector.reciprocal(rden, dps[:, 0:1])
            ops = out_ps.tile([T, E], F32, tag="ops")
            nc.tensor.matmul(ops, e_sb[b], outs16[b], start=True, stop=True)
            res = work.tile([T, E], F32, tag="res")
            nc.vector.tensor_scalar_mul(res, ops, rden)
            nc.sync.dma_start(out_d[b], res)

        def slab_addtanh(b, g, sbase, scount, big):
            for m in range(2):
                in0 = (whdup[b][:, m, 2 * sbase:2 * (sbase + scount)]
                       .rearrange("p (s two) -> p s two", two=2)
                       [:, :, None, :].broadcast_to([128, scount, 32, 2]))
                in1 = (wsT[:, m, b * T:(b + 1) * T]
                       .rearrange("p (t2 two) -> p t2 two", two=2)
                       [:, None, :, :].broadcast_to([128, scount, 32, 2]))
                oap = big[:, m, 0:64 * scount].rearrange(
                    "p (s t2 two) -> p s t2 two", s=scount, two=2)
                nc.vector.tensor_tensor(oap, in0, in1, op=mybir.AluOpType.add)
                nc.scalar.activation(big[:, m, 0:64 * scount],
                                     big[:, m, 0:64 * scount],
                                     mybir.ActivationFunctionType.Tanh)

        def slab_reduce(scount, big, scq):
            # m-outer: the m=0 pass streams while tanh m=1 still runs.
            # chunk k (8 s) -> psum rows 32*(k//2), col half 512*(k%2)
            for m in range(2):
                for k in range(scount // 8):
                    r, ch = 32 * (k // 2), 512 * (k % 2)
                    nc.tensor.matmul(scq[r:r + 32, ch:ch + 512],
                                     vrep[:, m, :],
                                     big[:, m, 512 * k:512 * (k + 1)],
                                     start=(m == 0), stop=(m == 1),
                                     skip_group_check=True)

        def emit_fill(n=1):
            if no_fill:
                return
            for _ in range(n):
                fill = out_ps.tile([T, 512], F32, tag="den")
                nc.tensor.matmul(fill, warmt[:, 0:T], warmt,
                                 start=True, stop=True, skip_group_check=True)

        # bridge the PE gap between fc and the first reduction (HAM would
        # re-throttle after 3.4us idle)
        emit_fill(8)

        def flush(pb, pg, psb, psc, pscq, evict_act=False):
            k1 = (psc // 8 + 1) // 2     # row groups (3 for 48, 2 for 32)
            scr = smallp.tile([96, 1024], F16, tag="scr")
            if evict_act:
                # tail flushes: ACT is idle after the last tanh, so its
                # copy runs in parallel with DVE's other evict
                nc.scalar.copy(scr[0:32 * k1, :], pscq[0:32 * k1, :])
            else:
                nc.vector.tensor_copy(scr[0:32 * k1, :], pscq[0:32 * k1, :])
            # flat dest (one partition dim); src flatten (k1,k2,slo,t)
            # matches dest s-local = 16*k1 + 8*k2 + slo
            src = scr[0:32 * k1:32, :].rearrange(
                "k1 (k2 slo t) -> k1 k2 slo t", k2=2, slo=8)
            nc.sync.dma_start(scoresT[pb][psb:psb + psc, :], src)
            # masked exp once rows are in place: after g1 the whole [0:96)
            # block (g0+g1) unconditionally -- skipped g1 rows hold stale
            # finite scores that the -1e30 mask bias zeroes; after g2 the
            # [96:128) block under its own If.
            if pg == 1:
                rows = slice(0, 96)
                nc.scalar.activation(e_sb[pb][rows, :], scoresT[pb][rows, :],
                                     mybir.ActivationFunctionType.Exp,
                                     bias=cb_sb[rows, 4 + pb:5 + pb])
            elif pg == 2:
                rows = slice(96, 128)
                if no_if:
                    nc.scalar.activation(e_sb[pb][rows, :],
                                         scoresT[pb][rows, :],
                                         mybir.ActivationFunctionType.Exp,
                                         bias=cb_sb[rows, 4 + pb:5 + pb])
                else:
                    with tc.If(rv_e[pb] > 96):
                        nc.scalar.activation(e_sb[pb][rows, :],
                                             scoresT[pb][rows, :],
                                             mybir.ActivationFunctionType.Exp,
                                             bias=cb_sb[rows, 4 + pb:5 + pb])
                emit_finals(pb)

        pending = []
        for i, (b, g) in enumerate(order):
            sbase, scount = sbases[g], SGROUPS[g]
            big = bigp.tile([128, 2, 64 * SGROUPS[0]], F16, tag="big")
            # adds+tanh first so they never queue behind the evict on DVE
            if g == 0 or no_if:
                slab_addtanh(b, g, sbase, scount, big)
            else:
                with tc.If(rv[b] > sbase):
                    slab_addtanh(b, g, sbase, scount, big)
            # flush slab i-2: its reduction finished ~2 slabs ago, and the
            # scq ring slot reader is known before slab i reallocates it
            if len(pending) >= 2:
                flush(*pending.pop(0))
            scq = scq_ps.tile([96, 1024], F32, tag="scq", name=f"scq{b}g{g}")
            if g == 0 or no_if:
                slab_reduce(scount, big, scq)
            else:
                with tc.If(rv_p[b] > sbase):
                    slab_reduce(scount, big, scq)
            emit_fill(1)
            pending.append((b, g, sbase, scount, scq))
        flush(*pending[0], evict_act=True)
        flush(*pending[1])

    nc.finalize()
    return nc


def _get_program():
    if "nc" not in _prog_cache:
        _prog_cache["nc"] = build_program()
    return _prog_cache["nc"]


def _balanced_assignment(src_len):
    """Pair batches so per-core work is balanced (min-max coverage).

    Returns perm: perm[c*BPC+j] = original batch index at core c slot j.
    Slot 1 (the tail-exposed batch) gets the smaller coverage of the pair.
    """
    bounds = np.cumsum([0] + SGROUPS)
    cov = [int(bounds[np.searchsorted(bounds, l)]) for l in src_len]
    order = sorted(range(B), key=lambda i: -cov[i])
    loads = [0] * NCORES
    slots = [[] for _ in range(NCORES)]
    for i in order:
        c = min((k for k in range(NCORES) if len(slots[k]) < BPC),
                key=lambda k: loads[k])
        slots[c].append(i)
        loads[c] += cov[i]
    for sl in slots:
        sl.sort(key=lambda i: -cov[i])   # slot order: [bigger, smaller]
    return [i for sl in slots for i in sl]


def make_in_maps(outputs, src_len, ss, W1, b1, W2, b2, v_w, v_b):
    outputs = np.asarray(outputs, dtype=np.float32)
    ss = np.asarray(ss, dtype=np.float32)
    src_len = np.asarray(src_len).astype(np.int64)
    perm = _balanced_assignment(src_len)
    maskb = np.where(np.arange(S)[None, :] < src_len[:, None],
                     np.float32(0.0), np.float32(-1e30)).astype(np.float32)

    # shared weight blobs
    w1b = np.zeros((128, W1B_COLS), dtype=np.float16)
    w1b[:, 0:1024] = np.asarray(W1, np.float16).reshape(4, 128, H) \
        .transpose(1, 0, 2).reshape(128, 1024)
    vr = np.repeat(np.asarray(v_w, np.float16).reshape(2, 128)
                   .transpose(1, 0)[:, :, None], 32, axis=2)   # [128, 2, 32]
    w1b[:, W1B_VREP:W1B_VREP + 64] = vr.reshape(128, 64)
    w1b[:, W1B_ONES] = np.float16(1.0)
    w2b = np.ascontiguousarray(np.asarray(W2, np.float16).reshape(2, 128, H)
                               .transpose(1, 0, 2).reshape(128, 512))

    b1_32 = np.asarray(b1, np.float32).reshape(2, 128).T     # [128, 2]
    b2_32 = np.asarray(b2, np.float32).reshape(2, 128).T

    oT16 = np.ascontiguousarray(
        outputs.transpose(1, 2, 0).astype(np.float16))        # (B, E, S)
    o16 = np.ascontiguousarray(
        outputs.transpose(1, 0, 2).astype(np.float16))        # (B, S, E)
    sT16 = np.ascontiguousarray(ss.transpose(1, 2, 0).astype(np.float16))  # (B,H,T)

    in_maps = []
    for c in range(NCORES):
        idx = perm[c * BPC:(c + 1) * BPC]
        cb = np.zeros((128, CB_COLS), dtype=np.float32)
        cb[:, 0:2] = b1_32
        cb[:, 2:4] = b2_32
        for j, bi in enumerate(idx):
            cb[:, 4 + j] = maskb[bi]
        # acts0: [e%128, c, b, s]
        a0 = oT16[idx].reshape(BPC, 4, 128, S).transpose(2, 1, 0, 3)
        # acts1: outs16 per batch [s, e]; ssb: ssT [h%128, c, (b,t)]
        a1 = np.zeros((128, BPC * E), dtype=np.float16)
        for j, bi in enumerate(idx):
            a1[:, j * E:(j + 1) * E] = o16[bi]
        ssb = np.ascontiguousarray(
            sT16[idx].reshape(BPC, 2, 128, T)
            .transpose(2, 1, 0, 3).reshape(128, 2 * BPC * T))
        in_maps.append({
            "slen": np.ascontiguousarray(src_len[idx].astype(np.int32)),
            "w1b": w1b,
            "w2b": w2b,
            "constsB": cb,
            "acts0": np.ascontiguousarray(a0),
            "acts1": a1,
            "ssb": ssb,
        })
    return in_maps, perm


def run(in_maps, trace=False, **kw):
    nc = _get_program()
    return run_bass_kernel_spmd(nc, in_maps, list(range(NCORES)), trace=trace, **kw)


def kernel(outputs, src_len, ss, W1, b1, W2, b2, v_w, v_b):
    in_maps, perm = make_in_maps(outputs, src_len, ss, W1, b1, W2, b2, v_w, v_b)
    res = run(in_maps)
    shuffled = np.concatenate([np.asarray(r["out"]).reshape(BPC, T, E)
                               for r in res.results], axis=0)
    out = np.empty_like(shuffled)
    out[np.asarray(perm)] = shuffled
    return out.astype(np.float32)  # (B, T, 2H)
